# revision 1
# baseline (speedup 1.0000x reference)
"""Trainium2 Bass kernel for CrossAttention (b=2, n=m=2048, dim=1024, 16 heads x 64).

Sharding: 8 cores = (2 batches) x (4 head-groups of 4 heads). Each core computes
q/k/v projections for its 4 heads, rotary, attention, and a partial output
projection y_part = O_heads @ Wo[head_rows]; host sums the 4 partials per batch
and adds bo.

Device-side layout trick: everything is computed transposed (features on
partitions) so no on-device transposes are needed anywhere:
  qT/kT [d(=64*2 per tile), n]  <- Wq^T @ x^T     (lhsT=Wq slice, rhs=x^T)
  S^T_j [128 ctx-tok, n-chunk]  <- k_j as lhsT, qT as rhs
  U = exp(S^T * scale)          (ScalarE, PSUM->SBUF bf16)
  O'^T/s  accumulate [65, n-chunk] <- lhsT=[v_j | 1], rhs=U  (sum row is free)
  O^T = O'^T * (1/s)            (VectorE, broadcast over partitions)
  y = (O^T).T @ Wo_rows         (lhsT=O^T tile, rhs=Wo rows)
Rotary pair-swap is a 32-lane stream_shuffle on VectorE; the +/- sign pattern is
folded into the precomputed sin table (host side).
Masks are all-True for this problem's input spec -> softmax is unmasked.
"""

import functools

import numpy as np
import ml_dtypes

import jax
from jax.experimental.shard_map import shard_map
from jax.sharding import Mesh, PartitionSpec

import concourse.bass as bass
import concourse.tile as tile
from concourse import bacc, bass2jax, mybir
from concourse.bass2jax import _bass_exec_p, install_neuronx_cc_hook

BF16 = ml_dtypes.bfloat16

B, N, DIM = 2, 2048, 1024
HEADS, DH = 16, 64
G = 4               # heads per core
N_CORES = 8
SCALE = DH ** -0.5
KSUB = DIM // 128   # 8
NT = N // 128       # 16 token tiles
SWAP_MASK = [i ^ 1 for i in range(32)]

_cached = {}


def _build_program(reps=1):
    """Build the SPMD Bass/Tile program (identical on all 8 cores).

    reps>1 repeats the whole computation (including input DMAs) for
    wall-clock benchmarking: per-iteration time = (wall_R - wall_1)/(R-1),
    which cancels the large axon dispatch/transfer overheads.
    """
    fp32 = mybir.dt.float32
    bf16 = mybir.dt.bfloat16
    EXP = mybir.ActivationFunctionType.Exp

    nc = bacc.Bacc("TRN2", target_bir_lowering=False, debug=False)

    xT_d = nc.dram_tensor("xT", [128, KSUB, N], bf16, kind="ExternalInput")
    cT_d = nc.dram_tensor("ctxT", [128, KSUB, N], bf16, kind="ExternalInput")
    wq_d = nc.dram_tensor("wq", [128, KSUB, 2 * 128], bf16, kind="ExternalInput")
    wk_d = nc.dram_tensor("wk", [128, KSUB, 2 * 128], bf16, kind="ExternalInput")
    wv_d = nc.dram_tensor("wv", [128, KSUB, 2 * 128], bf16, kind="ExternalInput")
    wo_d = nc.dram_tensor("wo", [128, 2, DIM], bf16, kind="ExternalInput")
    cos_d = nc.dram_tensor("cosT", [128, N], fp32, kind="ExternalInput")
    sin_d = nc.dram_tensor("sinT", [128, N], fp32, kind="ExternalInput")
    y_d = nc.dram_tensor("y", [NT, 128, DIM], fp32, kind="ExternalOutput")

    with tile.TileContext(nc) as tc:
        with (
            tc.tile_pool(name="consts", bufs=1) as consts,
            tc.tile_pool(name="ps", bufs=3, space="PSUM") as ps,
            tc.tile_pool(name="pop", bufs=2, space="PSUM") as pop,
            tc.tile_pool(name="ftmp", bufs=2) as ftmp,
            tc.tile_pool(name="upool", bufs=12) as upool,
            tc.tile_pool(name="ypool", bufs=4) as ypool,
            tc.tile_pool(name="rpool", bufs=4) as rpool,
        ):
          for _rep in range(reps):
            # ---- load constants / inputs into SBUF
            # small tensors first, then big activations chunked per-ksub so
            # compute can start as soon as the first chunks land.
            wv = consts.tile([128, KSUB, 256], bf16)
            wk = consts.tile([128, KSUB, 256], bf16)
            wq = consts.tile([128, KSUB, 256], bf16)
            wo = consts.tile([128, 2, DIM], bf16)
            cosT = consts.tile([128, N], fp32)
            sinT = consts.tile([128, N], fp32)
            xT = consts.tile([128, KSUB, N], bf16)
            ctxT = consts.tile([128, KSUB, N], bf16)
            nc.sync.dma_start(wv[:], wv_d[:])
            nc.sync.dma_start(ctxT[:, 0, :], cT_d[:, 0, :])
            nc.sync.dma_start(ctxT[:, 1, :], cT_d[:, 1, :])
            nc.sync.dma_start(wk[:], wk_d[:])
            nc.sync.dma_start(wq[:], wq_d[:])
            for ks in range(2, KSUB):
                nc.sync.dma_start(ctxT[:, ks, :], cT_d[:, ks, :])
            nc.sync.dma_start(cosT[:], cos_d[:])
            nc.sync.dma_start(sinT[:], sin_d[:])
            for ks in range(KSUB):
                nc.sync.dma_start(xT[:, ks, :], xT_d[:, ks, :])
            nc.sync.dma_start(wo[:], wo_d[:])

            # [part, head, ctx-tile, 64 v-dims + ones column]
            v_sb = consts.tile([128, G, NT, DH + 1], bf16)
            nc.gpsimd.memset(v_sb[:], 1.0)

            qrot = consts.tile([128, 2, N], bf16)   # [p, head-pair, n]
            krot = consts.tile([128, 2, N], bf16)
            ocat = consts.tile([128, 2, N], bf16)

            # ---- v projection (natural layout [ctx-tok, head-dims])
            def v_proj(jt):
                pv = ps.tile([128, 256], fp32, tag="ps", name="pv")
                for ks in range(KSUB):
                    nc.tensor.matmul(
                        pv[:], ctxT[:, ks, jt * 128:(jt + 1) * 128], wv[:, ks, :],
                        start=(ks == 0), stop=(ks == KSUB - 1),
                    )
                nc.vector.tensor_copy(
                    v_sb[:, :, jt, 0:DH],
                    pv[:].rearrange("p (h d) -> p h d", h=G),
                )

            # ---- q/k projections (transposed out) + rotary
            def proj_units(w_sb, src, rot, hp, c2):
                """Emission units (one per ksub + rotary tail) for weaving."""
                box = {}

                def mm(ks, c5):
                    if ks == 0 and c5 == 0:
                        box["pj"] = ps.tile([128, 1024], fp32, tag="ps",
                                            name="pj")
                    pj = box["pj"]
                    nc.tensor.matmul(
                        pj[:, c5 * 512:(c5 + 1) * 512],
                        w_sb[:, ks, hp * 128:(hp + 1) * 128],
                        src[:, ks, c2 * 1024 + c5 * 512:
                            c2 * 1024 + (c5 + 1) * 512],
                        start=(ks == 0), stop=(ks == KSUB - 1),
                    )

                def rotary():
                    pj = box["pj"]
                    nsl = slice(c2 * 1024, (c2 + 1) * 1024)
                    t1 = ftmp.tile([128, 1024], fp32, tag="t1", name="t1")
                    t2 = ftmp.tile([128, 1024], fp32, tag="t2", name="t2")
                    nc.vector.tensor_mul(t1[:], pj[:], cosT[:, nsl])
                    nc.vector.stream_shuffle(t2[:], pj[:], SWAP_MASK)
                    nc.vector.tensor_mul(t2[:], t2[:], sinT[:, nsl])
                    nc.vector.tensor_add(rot[:, hp, nsl], t1[:], t2[:])

                return [functools.partial(mm, ks, c5)
                        for ks in range(KSUB) for c5 in range(2)] + [rotary]

            def proj(w_sb, src, rot, hp, c2):
                for u in proj_units(w_sb, src, rot, hp, c2):
                    u()

            # ---- attention per (query-half, head); y projection for a
            # query-half is interleaved into the NEXT half's attention so its
            # PE work fills the ACT-bound slack there.
            def y_units(t):
                box = {}

                def mm(hp, c5):
                    if hp == 0 and c5 == 0:
                        box["py"] = ps.tile([128, 1024], fp32, tag="ps",
                                            name="py")
                    py = box["py"]
                    nc.tensor.matmul(
                        py[:, c5 * 512:(c5 + 1) * 512],
                        ocat[:, hp, t * 128:(t + 1) * 128],
                        wo[:, hp, c5 * 512:(c5 + 1) * 512],
                        start=(hp == 0), stop=(hp == 1),
                    )

                def out():
                    py = box["py"]
                    ysb = ypool.tile([128, 1024], fp32, tag="ysb", name="ysb")
                    nc.vector.tensor_copy(ysb[:, 0:512], py[:, 0:512])
                    nc.scalar.copy(ysb[:, 512:1024], py[:, 512:1024])
                    nc.sync.dma_start(y_d[t], ysb[:])

                return [functools.partial(mm, hp, c5)
                        for hp in range(2) for c5 in range(2)] + [out]

            def y_tile(t):
                for u in y_units(t):
                    u()

            import collections
            filler = collections.deque()

            def attn(hp, c4, budget=1):
                """Attention for the head PAIR hp (rows 0-63 / 64-127 of the
                qrot/krot tiles), query chunk c4 (512 wide). The two heads'
                S^T_j matmuls run concurrently in distinct PE row groups and
                write adjacent bank-halves of one PSUM tile, so a single
                FD=1024 exp covers both."""
                qsl = slice(c4 * 512, (c4 + 1) * 512)
                po = [pop.tile([DH + 1, 512], fp32, tag="po", name="po")
                      for _ in range(2)]
                for j in range(NT):
                    for _ in range(budget):
                        if filler:
                            filler.popleft()()
                    sps = ps.tile([128, 1024], fp32, tag="ps", name="sps")
                    for hh in range(2):
                        r = hh * 64
                        nc.tensor.matmul(
                            sps[:, hh * 512:(hh + 1) * 512],
                            krot[r:r + 64, hp, j * 128:(j + 1) * 128],
                            qrot[r:r + 64, hp, qsl],
                            start=True, stop=True, tile_position=(r, 0),
                        )
                    u = upool.tile([128, 1024], bf16, tag="u", name="u")
                    nc.scalar.activation(u[:], sps[:], EXP, scale=SCALE)
                    for hh in range(2):
                        nc.tensor.matmul(
                            po[hh][:],
                            v_sb[:, 2 * hp + hh, j, :],
                            u[:, hh * 512:(hh + 1) * 512],
                            start=(j == 0), stop=(j == NT - 1),
                        )
                with tc.high_priority(offset=120):
                    for hh in range(2):
                        r = hh * 64
                        rec = rpool.tile([1, 512], fp32, tag="rec", name="rec")
                        nc.vector.reciprocal(rec[:], po[hh][DH:DH + 1, :])
                        rec64 = rpool.tile([DH, 512], fp32, tag="rec64",
                                           name="rec64")
                        nc.gpsimd.partition_broadcast(rec64[:], rec[:])
                        nc.vector.tensor_tensor(
                            ocat[r:r + 64, hp, qsl],
                            po[hh][0:DH, :],
                            rec64[:],
                            mybir.AluOpType.mult,
                        )

            # weave: minimal prefix before the first attention head, then the
            # remaining projection / v / y work fills ACT-bound slack of the
            # already-running attention pipeline.
            # NOTE on dependencies: attn(h, c2) reads the FULL context range of
            # krot[hp(h)] (all j tiles) but only query-half c2 of qrot[hp(h)].
            # So both context-halves of a k projection must be fully emitted
            # (via the filler queue) before any head of that pair runs; q
            # projections per query-half likewise before their consumers. The
            # filler queue drains one unit per attention j-step, 16 units per
            # head, so the placement below guarantees: k(hp1) drains within
            # attn(0,0)+attn(1,0) (32 slots >= 18 units) before attn(2,0);
            # q(1,0) before attn(2,0); q(0,1)/q(1,1) before attn(*,1).
            # prefix: minimal work before the first attention pair can start:
            # a few v tiles + hp0's k (both ctx halves) + q (first query half)
            for jt in range(2):
                v_proj(jt)
            proj(wk, ctxT, krot, 0, 0)
            for jt in range(2, 4):
                v_proj(jt)
            proj(wk, ctxT, krot, 0, 1)
            for jt in range(4, 8):
                v_proj(jt)
            proj(wq, xT, qrot, 0, 0)
            for jt in range(8, NT):
                v_proj(jt)
            filler.extend(proj_units(wk, ctxT, krot, 1, 0))
            filler.extend(proj_units(wk, ctxT, krot, 1, 1))
            attn(0, 0, budget=2)
            filler.extend(proj_units(wq, xT, qrot, 1, 0))
            attn(0, 1, budget=1)
            while filler:   # k(hp1) + q(1,0) fully emitted
                filler.popleft()()
            filler.extend(proj_units(wq, xT, qrot, 0, 1))
            attn(1, 0, budget=1)
            filler.extend(proj_units(wq, xT, qrot, 1, 1))
            attn(1, 1, budget=1)
            while filler:   # q(0,1) + q(1,1) fully emitted
                filler.popleft()()
            # query chunks 2-3; weave y tiles as soon as their token range is
            # final: t 0..7 after chunks 0-1, t 8..11 (tokens 1024..1535)
            # after chunk 2 — leaving only y(12..15) past the last attention.
            for t in range(0, 8):
                filler.extend(y_units(t))
            attn(0, 2, budget=2)
            attn(1, 2, budget=2)
            while filler:
                filler.popleft()()
            for t in range(8, 12):
                filler.extend(y_units(t))
            attn(0, 3, budget=1)
            attn(1, 3, budget=1)
            while filler:
                filler.popleft()()
            for t in range(12, NT):
                y_tile(t)

    nc.finalize()
    return nc


def _prep_inputs(x, context, rotary_pos, Wq, Wkv, Wo):
    """Build the 8 per-core input maps (host-side shard + transpose + cast)."""
    x = np.asarray(x, dtype=np.float32)
    context = np.asarray(context, dtype=np.float32)
    rotary_pos = np.asarray(rotary_pos, dtype=np.float32)
    Wq = np.asarray(Wq, dtype=np.float32)
    Wkv = np.asarray(Wkv, dtype=np.float32)
    Wo = np.asarray(Wo, dtype=np.float32)

    Wk, Wv = Wkv[:, :DIM], Wkv[:, DIM:]

    cos = np.cos(rotary_pos).T.astype(np.float32)                # [64, n]
    sign = np.tile(np.array([-1.0, 1.0], np.float32), DH // 2)   # rotate_half sign
    sin = (np.sin(rotary_pos) * sign[None, :]).T.astype(np.float32)
    cosT = np.ascontiguousarray(np.concatenate([cos, cos], axis=0))   # [128, n]
    sinT = np.ascontiguousarray(np.concatenate([sin, sin], axis=0))

    def to_kxm(w):  # [1024, 256] -> [128, 8, 256] (partition, ksub, m)
        return np.ascontiguousarray(
            w.reshape(KSUB, 128, w.shape[1]).transpose(1, 0, 2).astype(BF16))

    def to_pT(a):   # [2048, 1024] -> [128, 8, 2048]
        return np.ascontiguousarray(
            a.T.reshape(KSUB, 128, N).transpose(1, 0, 2).astype(BF16))

    in_maps = []
    for core in range(N_CORES):
        b, g = divmod(core, G)
        cs = slice(g * G * DH, (g + 1) * G * DH)   # 256 cols of this head group
        in_maps.append({
            "xT": to_pT(x[b]),
            "ctxT": to_pT(context[b]),
            "wq": to_kxm(Wq[:, cs]),
            "wk": to_kxm(Wk[:, cs]),
            "wv": to_kxm(Wv[:, cs]),
            "wo": np.ascontiguousarray(
                Wo[cs, :].reshape(2, 128, DIM).transpose(1, 0, 2).astype(BF16)),
            "cosT": cosT,
            "sinT": sinT,
        })
    return in_maps


def _ensure_runner(reps=1):
    """Build the Bass program and a reusable jitted SPMD executor.

    Returns (exec_fn, in_names, out_info): exec_fn(concat_inputs) -> concat
    output arrays (blocking); concat_inputs are the per-core input arrays
    concatenated along axis 0 in in_names order.
    """
    key = ("runner", reps)
    if key in _cached:
        return _cached[key]

    nc = _build_program(reps=reps)
    install_neuronx_cc_hook()
    partition_name = nc.partition_id_tensor.name if nc.partition_id_tensor else None

    in_names, out_names, out_avals = [], [], []
    for alloc in nc.m.functions[0].allocations:
        if not isinstance(alloc, mybir.MemoryLocationSet):
            continue
        name = alloc.memorylocations[0].name
        if alloc.kind == "ExternalInput":
            if name != partition_name:
                in_names.append(name)
        elif alloc.kind == "ExternalOutput":
            out_names.append(name)
            out_avals.append(jax.core.ShapedArray(
                tuple(alloc.tensor_shape), mybir.dt.np(alloc.dtype)))
    n_params = len(in_names)
    all_in_names = list(in_names) + list(out_names)
    if partition_name is not None:
        all_in_names.append(partition_name)

    def _body(*args):
        operands = list(args)
        if partition_name is not None:
            operands.append(bass2jax.partition_id_tensor())
        return tuple(_bass_exec_p.bind(
            *operands,
            out_avals=tuple(out_avals),
            in_names=tuple(all_in_names),
            out_names=tuple(out_names),
            lowering_input_output_aliases=(),
            sim_require_finite=True,
            sim_require_nnan=True,
            nc=nc,
        ))

    devices = jax.devices()[:N_CORES]
    mesh = Mesh(np.asarray(devices), ("core",))
    n_outs = len(out_names)
    donate = tuple(range(n_params, n_params + n_outs))
    sharded = jax.jit(
        shard_map(_body, mesh=mesh,
                  in_specs=(PartitionSpec("core"),) * (n_params + n_outs),
                  out_specs=(PartitionSpec("core"),) * n_outs,
                  check_rep=False),
        donate_argnums=donate,
        keep_unused=True,
    )

    import jax.numpy as jnp
    from jax.sharding import NamedSharding

    zero_shardings = tuple(
        NamedSharding(mesh, PartitionSpec("core")) for _ in out_avals)

    @functools.partial(jax.jit, out_shardings=zero_shardings)
    def zmaker():
        return tuple(
            jnp.zeros((N_CORES * a.shape[0], *a.shape[1:]), a.dtype)
            for a in out_avals)

    def exec_fn(concat_in):
        zeros = zmaker()
        outs = sharded(*concat_in, *zeros)
        jax.block_until_ready(outs)
        return outs

    _cached[key] = (exec_fn, in_names, out_names, out_avals,
                    sharded, zmaker)
    return _cached[key]


def _concat_inputs(in_maps, in_names):
    return [
        np.concatenate([np.asarray(in_maps[c][name]) for c in range(N_CORES)],
                       axis=0)
        for name in in_names
    ]


def _run(inputs, trace=False):
    exec_fn, in_names, out_names, out_avals = _ensure_runner()[:4]
    in_maps = _prep_inputs(
        inputs["x"], inputs["context"], inputs["rotary_pos"],
        inputs["Wq"], inputs["Wkv"], inputs["Wo"])
    outs = exec_fn(_concat_inputs(in_maps, in_names))

    yi = out_names.index("y")
    y_all = np.asarray(outs[yi]).reshape(N_CORES, *out_avals[yi].shape)

    bo = np.asarray(inputs["bo"], dtype=np.float32)
    y = np.zeros((B, N, DIM), dtype=np.float32)
    for core in range(N_CORES):
        y[core // G] += y_all[core].reshape(N, DIM)
    y += bo[None, None, :]
    return y, None


def kernel(**inputs) -> np.ndarray:
    y, _ = _run(inputs, trace=False)
    return y



# revision 17
# speedup vs baseline: 1.1749x; 1.1749x over previous
"""Trainium2 Bass kernel for CrossAttention (b=2, n=m=2048, dim=1024, 16 heads x 64).

Sharding: 8 cores = (2 batches) x (4 head-groups of 4 heads). Each core computes
q/k/v projections for its 4 heads, rotary, attention, and a partial output
projection y_part = O_heads @ Wo[head_rows]; host sums the 4 partials per batch
and adds bo.

Device-side layout (everything transposed, features on partitions — no
on-device transposes needed):
  qT/kT [d(=64*2 per tile), n]  <- Wq^T @ x^T     (lhsT=Wq slice, rhs=x^T)
  S^T_j [128 ctx-tok, n-chunk]  <- k_j as lhsT, qT as rhs
  U = exp(S^T * scale)          (ScalarE, PSUM->SBUF bf16)
  O'^T/s  accumulate [65, n-chunk] <- lhsT=[v_j | 1], rhs=U  (sum row is free)
  O^T = O'^T * (1/s)            (VectorE, broadcast over partitions)
  y = (O^T).T @ Wo_rows         (lhsT=O^T tile, rhs=Wo rows)
Rotary pair-swap is a 32-lane stream_shuffle on VectorE; the +/- sign pattern is
folded into the precomputed sin table (host side).
Masks are all-True for this problem's input spec -> softmax is unmasked.

Schedule (v2): PE is the binding engine (~164us of matmul rows); ACT is next
(~134us of exp).  The design keeps PE saturated end-to-end:
  - ctx/x stream in POSITION slabs over three DMA queues (SP/Pool/ACT DGE),
    so krot/qrot tiles become ready incrementally and attention starts ~9us in.
  - dedicated PSUM rings: sps 2x[128,1024] (score tiles), pj 4x[128,256]
    (all projection chunks + v tiles + y quarters), po 2x[65,512] (av
    accumulators) — projection/rotary latency never blocks the score ring.
  - all projections are 256-col chunks (8 accumulating matmuls + DVE rotary)
    emitted as filler groups inside the attention j-loops at a fixed budget.
  - y tiles are computed in 256-wide quarters, copied PSUM->SBUF on gpsimd
    (keeps ScalarE free for exp) and written out from the Pool DGE queue so
    rep i+1's input DMAs (SP queue) prefetch during rep i's attention.
"""

import collections
import functools

import numpy as np
import ml_dtypes

import jax
from jax.experimental.shard_map import shard_map
from jax.sharding import Mesh, PartitionSpec

import concourse.bass as bass
import concourse.tile as tile
from concourse import bacc, bass2jax, mybir
from concourse.bass2jax import _bass_exec_p, install_neuronx_cc_hook

BF16 = ml_dtypes.bfloat16

B, N, DIM = 2, 2048, 1024
HEADS, DH = 16, 64
G = 4               # heads per core
N_CORES = 8
SCALE = DH ** -0.5
KSUB = DIM // 128   # 8
NT = N // 128       # 16 token tiles
SWAP_MASK = [i ^ 1 for i in range(32)]

_cached = {}


def _build_program(reps=1):
    """Build the SPMD Bass/Tile program (identical on all 8 cores).

    reps>1 repeats the whole computation (including input DMAs) for
    wall-clock benchmarking: per-iteration time = (wall_R - wall_1)/(R-1),
    which cancels the large axon dispatch/transfer overheads.
    """
    fp32 = mybir.dt.float32
    bf16 = mybir.dt.bfloat16
    EXP = mybir.ActivationFunctionType.Exp

    nc = bacc.Bacc("TRN2", target_bir_lowering=False, debug=False)

    xT_d = nc.dram_tensor("xT", [128, KSUB, N], bf16, kind="ExternalInput")
    cT_d = nc.dram_tensor("ctxT", [128, KSUB, N], bf16, kind="ExternalInput")
    wq_d = nc.dram_tensor("wq", [128, KSUB, 2 * 128], bf16, kind="ExternalInput")
    wk_d = nc.dram_tensor("wk", [128, KSUB, 2 * 128], bf16, kind="ExternalInput")
    wv_d = nc.dram_tensor("wv", [128, KSUB, 2 * 128], bf16, kind="ExternalInput")
    wo_d = nc.dram_tensor("wo", [128, 2, DIM], bf16, kind="ExternalInput")
    cos_d = nc.dram_tensor("cosT", [128, N], bf16, kind="ExternalInput")
    sin_d = nc.dram_tensor("sinT", [128, N], bf16, kind="ExternalInput")
    y_d = nc.dram_tensor("y", [NT, 128, DIM], fp32, kind="ExternalOutput")

    with tile.TileContext(nc) as tc:
        with (
            tc.tile_pool(name="consts", bufs=1) as consts,
            tc.tile_pool(name="dbl", bufs=2) as dbl,
            tc.tile_pool(name="psum", bufs=2, space="PSUM") as psum,
            tc.tile_pool(name="ftmp", bufs=4) as ftmp,
            tc.tile_pool(name="upool", bufs=10) as upool,
            tc.tile_pool(name="ypool", bufs=4) as ypool,
            tc.tile_pool(name="rpool", bufs=2) as rpool,
        ):
          for _rep in range(reps):
            # ---- SBUF tiles
            wq = consts.tile([128, KSUB, 256], bf16)
            wk = consts.tile([128, KSUB, 256], bf16)
            wv = consts.tile([128, KSUB, 256], bf16)
            wo = consts.tile([128, 2, DIM], bf16)
            cosT = consts.tile([128, N], bf16)
            sinT = consts.tile([128, N], bf16)
            xT = consts.tile([128, KSUB, N], bf16)
            ctxT = consts.tile([128, KSUB, N], bf16)
            ocat = consts.tile([128, 2, N], bf16)
            # cross-rep double buffered (their readers live until rep end)
            v_sb = dbl.tile([128, G, NT, DH + 1], bf16)   # [p, head, ctx-tile, v|1]
            qrot = dbl.tile([128, 2, N], bf16)            # [p, head-pair, n]
            krot = dbl.tile([128, 2, N], bf16)

            # ---- input DMAs: the sim (and HBM) serialize transfers, so a
            # single SP queue in NEED order beats parallel queues.  ctx
            # position-slabs and x column-slabs land just-in-time for the
            # k/q projection chunks that consume them; y outputs ride the
            # Pool DGE so rep i+1's input stream prefetches during rep i.
            def ctx_slab(p):
                sl = slice(p * 256, (p + 1) * 256)
                nc.sync.dma_start(ctxT[:, :, sl], cT_d[:, :, sl])

            def x_slab(c0, w):
                sl = slice(c0, c0 + w)
                nc.sync.dma_start(xT[:, :, sl], xT_d[:, :, sl])

            nc.sync.dma_start(wk[:], wk_d[:])
            ctx_slab(0)
            nc.sync.dma_start(wv[:], wv_d[:])
            ctx_slab(1)
            nc.sync.dma_start(cosT[:], cos_d[:])
            nc.sync.dma_start(sinT[:], sin_d[:])
            ctx_slab(2)
            ctx_slab(3)
            nc.sync.dma_start(wq[:], wq_d[:])
            x_slab(0, 512)
            for p in range(4, 8):
                ctx_slab(p)
            x_slab(512, 512)
            x_slab(1024, 512)
            x_slab(1536, 512)
            nc.sync.dma_start(wo[:], wo_d[:])

            nc.gpsimd.memset(v_sb[:, :, :, DH:DH + 1], 1.0)

            # ---- filler groups -------------------------------------------
            def proj_group(w_sb, src, rot, hp, c0):
                """One 256-col projection chunk: 8 accumulating matmuls into a
                pj-ring PSUM tile + rotary (2 muls, pair-swap shuffle, add)."""
                def g():
                    csl = slice(c0, c0 + 256)
                    pj = psum.tile([128, 256], fp32, tag="pj", name="pj",
                                   bufs=2)
                    for ks in range(KSUB):
                        nc.tensor.matmul(
                            pj[:], w_sb[:, ks, hp * 128:(hp + 1) * 128],
                            src[:, ks, csl],
                            start=(ks == 0), stop=(ks == KSUB - 1),
                        )
                    t1 = ftmp.tile([128, 256], fp32, tag="t1", name="t1")
                    t2 = ftmp.tile([128, 256], fp32, tag="t2", name="t2")
                    nc.vector.tensor_mul(t1[:], pj[:], cosT[:, csl])
                    nc.vector.stream_shuffle(t2[:], pj[:], SWAP_MASK)
                    nc.vector.tensor_mul(t2[:], t2[:], sinT[:, csl])
                    nc.vector.tensor_add(rot[:, hp, csl], t1[:], t2[:])
                return g

            def v_group(jt):
                """v projection for one 128-token ctx tile (natural layout)."""
                def g():
                    pv = psum.tile([128, 256], fp32, tag="pj", name="pv",
                                   bufs=2)
                    for ks in range(KSUB):
                        nc.tensor.matmul(
                            pv[:], ctxT[:, ks, jt * 128:(jt + 1) * 128],
                            wv[:, ks, :],
                            start=(ks == 0), stop=(ks == KSUB - 1),
                        )
                    nc.vector.tensor_copy(
                        v_sb[:, :, jt, 0:DH],
                        pv[:].rearrange("p (h d) -> p h d", h=G),
                    )
                return g

            ybox = {}

            TAGB = {"pj": 2, "po": 2, "sps": 2}

            def y_group(t, qtr, tag="pj", copy_eng=None, dma_eng=None):
                """One 256-wide quarter of output tile t: 2 accumulating
                matmuls (head-pair halves of the inner dim), gpsimd copy to
                SBUF; DMA out (Pool DGE) once all 4 quarters landed.  In the
                tail (no attention running) quarters cycle through all three
                PSUM tags so the mm->copy chain pipelines 6-wide."""
                def g():
                    if qtr == 0:
                        ybox[t] = ypool.tile([128, DIM], fp32, tag="ysb",
                                             name="ysb")
                    ysb = ybox[t]
                    qsl = slice(qtr * 256, (qtr + 1) * 256)
                    py = psum.tile([128, 256], fp32, tag=tag, name="py",
                                   bufs=TAGB[tag])
                    for hp in range(2):
                        nc.tensor.matmul(
                            py[:], ocat[:, hp, t * 128:(t + 1) * 128],
                            wo[:, hp, qsl],
                            start=(hp == 0), stop=(hp == 1),
                        )
                    eng = copy_eng or nc.vector
                    if eng is nc.scalar:
                        eng.copy(ysb[:, qsl], py[:])
                    else:
                        eng.tensor_copy(ysb[:, qsl], py[:])
                    if qtr == 3:
                        (dma_eng or nc.gpsimd).dma_start(y_d[t], ysb[:])
                        del ybox[t]
                return g

            filler = collections.deque()

            def attn(hp, c4, budget=1):
                """Attention for head PAIR hp, query chunk c4 (512 wide). The
                two heads' S^T_j matmuls run in distinct PE row groups and
                write adjacent bank-halves of one PSUM tile, so a single
                FD=1024 exp covers both."""
                qsl = slice(c4 * 512, (c4 + 1) * 512)
                po = [psum.tile([DH + 1, 512], fp32, tag="po", name="po",
                                bufs=2) for _ in range(2)]
                for j in range(NT):
                    for _ in range(budget):
                        if filler:
                            filler.popleft()()
                    sps = psum.tile([128, 1024], fp32, tag="sps", name="sps",
                                    bufs=2)
                    for hh in range(2):
                        r = hh * 64
                        nc.tensor.matmul(
                            sps[:, hh * 512:(hh + 1) * 512],
                            krot[r:r + 64, hp, j * 128:(j + 1) * 128],
                            qrot[r:r + 64, hp, qsl],
                            start=True, stop=True, tile_position=(r, 0),
                        )
                    u = upool.tile([128, 1024], bf16, tag="u", name="u")
                    nc.scalar.activation(u[:], sps[:], EXP, scale=SCALE)
                    for hh in range(2):
                        nc.tensor.matmul(
                            po[hh][:],
                            v_sb[:, 2 * hp + hh, j, :],
                            u[:, hh * 512:(hh + 1) * 512],
                            start=(j == 0), stop=(j == NT - 1),
                        )
                with tc.high_priority(offset=120):
                    for hh in range(2):
                        r = hh * 64
                        rec = rpool.tile([1, 512], fp32, tag="rec", name="rec")
                        nc.vector.reciprocal(rec[:], po[hh][DH:DH + 1, :])
                        rec64 = rpool.tile([DH, 512], fp32, tag="rec64",
                                           name="rec64")
                        nc.gpsimd.partition_broadcast(rec64[:], rec[:])
                        nc.vector.tensor_tensor(
                            ocat[r:r + 64, hp, qsl],
                            po[hh][0:DH, :],
                            rec64[:],
                            mybir.AluOpType.mult,
                        )

            # ---- schedule -----------------------------------------------
            # stream phase: consume ctx slabs 0-1 fully on arrival (k both
            # head-pairs + v tiles), then q(hp0, first 512 q) so A1 can start.
            for p in range(4):
                proj_group(wk, ctxT, krot, 0, p * 256)()
                proj_group(wk, ctxT, krot, 1, p * 256)()
                v_group(2 * p)()
                v_group(2 * p + 1)()
            proj_group(wq, xT, qrot, 0, 0)()
            proj_group(wq, xT, qrot, 0, 256)()

            # A1=(0,0): remaining ctx slabs (k both pairs + v, in slab-arrival
            # order, v one step ahead of its av consumer) + q(0, c4=1).
            for p in range(4, 8):
                filler.extend([proj_group(wk, ctxT, krot, 0, p * 256),
                               proj_group(wk, ctxT, krot, 1, p * 256),
                               v_group(2 * p), v_group(2 * p + 1)])
            filler.extend([proj_group(wq, xT, qrot, 0, 512),
                           proj_group(wq, xT, qrot, 0, 768)])
            attn(0, 0, budget=2)
            while filler:
                filler.popleft()()
            # A2=(0,1): q(hp1, c4 0-1)
            filler.extend([proj_group(wq, xT, qrot, 1, 0),
                           proj_group(wq, xT, qrot, 1, 256),
                           proj_group(wq, xT, qrot, 1, 512),
                           proj_group(wq, xT, qrot, 1, 768)])
            attn(0, 1, budget=1)
            while filler:
                filler.popleft()()
            # A3=(1,0): q(c4=2) both head-pairs.
            filler.extend([proj_group(wq, xT, qrot, 0, 1024),
                           proj_group(wq, xT, qrot, 0, 1280),
                           proj_group(wq, xT, qrot, 1, 1024),
                           proj_group(wq, xT, qrot, 1, 1280)])
            attn(1, 0, budget=1)
            while filler:
                filler.popleft()()
            # A4=(1,1): y(q 0:512) + q(c4=3) both head-pairs.
            filler.extend([y_group(t, q) for t in range(0, 4)
                           for q in range(4)]
                          + [proj_group(wq, xT, qrot, 0, 1536),
                             proj_group(wq, xT, qrot, 0, 1792),
                             proj_group(wq, xT, qrot, 1, 1536),
                             proj_group(wq, xT, qrot, 1, 1792)])
            attn(1, 1, budget=2)
            while filler:
                filler.popleft()()
            # A5=(0,2) / A6=(1,2): y(q 512:1024) split between them
            filler.extend([y_group(t, q) for t in range(4, 6)
                           for q in range(4)])
            attn(0, 2, budget=1)
            while filler:
                filler.popleft()()
            filler.extend([y_group(t, q) for t in range(6, 8)
                           for q in range(4)])
            attn(1, 2, budget=1)
            while filler:
                filler.popleft()()
            # A7=(0,3) / A8=(1,3): y(q 1024:1536) split between them
            filler.extend([y_group(t, q) for t in range(8, 10)
                           for q in range(4)])
            attn(0, 3, budget=1)
            while filler:
                filler.popleft()()
            filler.extend([y_group(t, q) for t in range(10, 12)
                           for q in range(4)])
            attn(1, 3, budget=1)
            while filler:
                filler.popleft()()
            # tail: y(q 1536:2048) — cycle PSUM tags for 6-wide pipelining
            tags = ["pj", "po", "sps"]
            i = 0
            for t in range(12, NT):
                for q in range(4):
                    eng = [nc.vector, nc.scalar][i % 2]
                    y_group(t, q, tag=tags[i % 3], copy_eng=eng,
                            dma_eng=nc.scalar)()
                    i += 1

    nc.finalize()
    return nc


def _prep_inputs(x, context, rotary_pos, Wq, Wkv, Wo):
    """Build the 8 per-core input maps (host-side shard + transpose + cast)."""
    x = np.asarray(x, dtype=np.float32)
    context = np.asarray(context, dtype=np.float32)
    rotary_pos = np.asarray(rotary_pos, dtype=np.float32)
    Wq = np.asarray(Wq, dtype=np.float32)
    Wkv = np.asarray(Wkv, dtype=np.float32)
    Wo = np.asarray(Wo, dtype=np.float32)

    Wk, Wv = Wkv[:, :DIM], Wkv[:, DIM:]

    cos = np.cos(rotary_pos).T.astype(np.float32)                # [64, n]
    sign = np.tile(np.array([-1.0, 1.0], np.float32), DH // 2)   # rotate_half sign
    sin = (np.sin(rotary_pos) * sign[None, :]).T.astype(np.float32)
    cosT = np.ascontiguousarray(np.concatenate([cos, cos], axis=0).astype(BF16))
    sinT = np.ascontiguousarray(np.concatenate([sin, sin], axis=0).astype(BF16))

    def to_kxm(w):  # [1024, 256] -> [128, 8, 256] (partition, ksub, m)
        return np.ascontiguousarray(
            w.reshape(KSUB, 128, w.shape[1]).transpose(1, 0, 2).astype(BF16))

    def to_pT(a):   # [2048, 1024] -> [128, 8, 2048]
        return np.ascontiguousarray(
            a.T.reshape(KSUB, 128, N).transpose(1, 0, 2).astype(BF16))

    in_maps = []
    for core in range(N_CORES):
        b, g = divmod(core, G)
        cs = slice(g * G * DH, (g + 1) * G * DH)   # 256 cols of this head group
        in_maps.append({
            "xT": to_pT(x[b]),
            "ctxT": to_pT(context[b]),
            "wq": to_kxm(Wq[:, cs]),
            "wk": to_kxm(Wk[:, cs]),
            "wv": to_kxm(Wv[:, cs]),
            "wo": np.ascontiguousarray(
                Wo[cs, :].reshape(2, 128, DIM).transpose(1, 0, 2).astype(BF16)),
            "cosT": cosT,
            "sinT": sinT,
        })
    return in_maps


def _ensure_runner(reps=1):
    """Build the Bass program and a reusable jitted SPMD executor.

    Returns (exec_fn, in_names, out_info): exec_fn(concat_inputs) -> concat
    output arrays (blocking); concat_inputs are the per-core input arrays
    concatenated along axis 0 in in_names order.
    """
    key = ("runner", reps)
    if key in _cached:
        return _cached[key]

    nc = _build_program(reps=reps)
    install_neuronx_cc_hook()
    partition_name = nc.partition_id_tensor.name if nc.partition_id_tensor else None

    in_names, out_names, out_avals = [], [], []
    for alloc in nc.m.functions[0].allocations:
        if not isinstance(alloc, mybir.MemoryLocationSet):
            continue
        name = alloc.memorylocations[0].name
        if alloc.kind == "ExternalInput":
            if name != partition_name:
                in_names.append(name)
        elif alloc.kind == "ExternalOutput":
            out_names.append(name)
            out_avals.append(jax.core.ShapedArray(
                tuple(alloc.tensor_shape), mybir.dt.np(alloc.dtype)))
    n_params = len(in_names)
    all_in_names = list(in_names) + list(out_names)
    if partition_name is not None:
        all_in_names.append(partition_name)

    def _body(*args):
        operands = list(args)
        if partition_name is not None:
            operands.append(bass2jax.partition_id_tensor())
        return tuple(_bass_exec_p.bind(
            *operands,
            out_avals=tuple(out_avals),
            in_names=tuple(all_in_names),
            out_names=tuple(out_names),
            lowering_input_output_aliases=(),
            sim_require_finite=True,
            sim_require_nnan=True,
            nc=nc,
        ))

    devices = jax.devices()[:N_CORES]
    mesh = Mesh(np.asarray(devices), ("core",))
    n_outs = len(out_names)
    donate = tuple(range(n_params, n_params + n_outs))
    sharded = jax.jit(
        shard_map(_body, mesh=mesh,
                  in_specs=(PartitionSpec("core"),) * (n_params + n_outs),
                  out_specs=(PartitionSpec("core"),) * n_outs,
                  check_rep=False),
        donate_argnums=donate,
        keep_unused=True,
    )

    import jax.numpy as jnp
    from jax.sharding import NamedSharding

    zero_shardings = tuple(
        NamedSharding(mesh, PartitionSpec("core")) for _ in out_avals)

    @functools.partial(jax.jit, out_shardings=zero_shardings)
    def zmaker():
        return tuple(
            jnp.zeros((N_CORES * a.shape[0], *a.shape[1:]), a.dtype)
            for a in out_avals)

    def exec_fn(concat_in):
        zeros = zmaker()
        outs = sharded(*concat_in, *zeros)
        jax.block_until_ready(outs)
        return outs

    _cached[key] = (exec_fn, in_names, out_names, out_avals,
                    sharded, zmaker)
    return _cached[key]


def _concat_inputs(in_maps, in_names):
    return [
        np.concatenate([np.asarray(in_maps[c][name]) for c in range(N_CORES)],
                       axis=0)
        for name in in_names
    ]


def _run(inputs, trace=False):
    exec_fn, in_names, out_names, out_avals = _ensure_runner()[:4]
    in_maps = _prep_inputs(
        inputs["x"], inputs["context"], inputs["rotary_pos"],
        inputs["Wq"], inputs["Wkv"], inputs["Wo"])
    outs = exec_fn(_concat_inputs(in_maps, in_names))

    yi = out_names.index("y")
    y_all = np.asarray(outs[yi]).reshape(N_CORES, *out_avals[yi].shape)

    bo = np.asarray(inputs["bo"], dtype=np.float32)
    y = np.zeros((B, N, DIM), dtype=np.float32)
    for core in range(N_CORES):
        y[core // G] += y_all[core].reshape(N, DIM).astype(np.float32)
    y += bo[None, None, :]
    return y, None


def kernel(**inputs) -> np.ndarray:
    y, _ = _run(inputs, trace=False)
    return y
